# revision 2
# baseline (speedup 1.0000x reference)
"""Trainium2 Bass kernel: CapOnlyContrastiveLoss (margin contrastive loss, mean reduction).

reference math (N=8192, D=512, margin=0.2):
    scores[i,j]  = -||im_i - ex_j||        (via gemm identity)
    diag[i]      = -||im_i - s_i||         (only the diagonal of l2_sim(im, s) is used)
    loss         = mean(relu(margin + scores - diag))

Strategy (v2, fp8 DoubleRow):
  * 4x2 core grid over (im rows, ex rows): each core handles 2048 im/s rows x
    4096 ex rows -> reads 16 MB of HBM.
  * PE: -2*im.ex via fp8(e4m3) DoubleRow matmuls: K=512 in 2 MMs of K=256
    (2x fewer streaming cycles than bf16).  Operands are produced by XBAR
    transposes of the bf16 view (high 16 bits) of the fp32 loads, then a DVE
    cast to fp8 (x -2 folded into the im-side cast).
  * exsq[j] folded into PSUM via one K=2 bf16 matmul per bank (ones x
    hi/lo bf16 split of exsq).  The 4 per-group exsq MMs use tile_position
    row groups 0/32/64/96 so they run concurrently on the PE array.
  * Epilogue per (jb,it) group = 4 PSUM banks at once:
      ACT: sq[128,2048] = sqrt(psum + imsq[i])        (one instr, fp16 out)
      DVE: acc[:,g]    += sum_j min(sq, c[i])          (one tensor_scalar with
           fused min+add accumulator), using relu(c-s) = c - min(s,c).
  * Host finishes: loss = sum_cores(EX_R * sum_i c_i - sum(acc)) / N^2.
"""

import numpy as np

import concourse.bacc as bacc
import concourse.bass as bass
import concourse.tile as tile
from concourse import bass_utils, mybir

N, D = 8192, 512
MARGIN = 0.2
P = 128
NJ = 512  # one PSUM bank of fp32
GW = 2048  # epilogue group width along j (4 PSUM banks)
I_GROUPS, J_GROUPS = 4, 2  # 8 cores
IM_R = N // I_GROUPS  # 2048 im/s rows per core
EX_R = N // J_GROUPS  # 4096 ex rows per core
KC = D // P  # 4 contraction chunks of 128
N_IT = IM_R // P  # 16 i tiles
N_JB = EX_R // GW  # 2 j groups
BANKS = GW // NJ  # 4 banks per group
EX_PER_JB = GW // P  # 16 ex row-tiles per j group
N_G = N_JB * N_IT  # 32 epilogue groups

F32 = mybir.dt.float32
BF16 = mybir.dt.bfloat16
FP16 = mybir.dt.float16
FP8 = mybir.dt.float8e4
AF = mybir.ActivationFunctionType
ALU = mybir.AluOpType
DR = mybir.MatmulPerfMode.DoubleRow

_CACHE = {}


def _emit(tc, nc, im_d, s_d, ex_d, acc_d, cvec_d):
    from contextlib import ExitStack

    with ExitStack() as ctx:
        singles = ctx.enter_context(tc.tile_pool(name="singles", bufs=1))
        loads = ctx.enter_context(tc.tile_pool(name="loads", bufs=8))
        casts = ctx.enter_context(tc.tile_pool(name="casts", bufs=6))
        tbuf = ctx.enter_context(tc.tile_pool(name="tbuf", bufs=6))
        scratch = ctx.enter_context(tc.tile_pool(name="scratch", bufs=8))
        sqp = ctx.enter_context(tc.tile_pool(name="sqp", bufs=3))
        psum = ctx.enter_context(tc.tile_pool(name="psum", bufs=2, space="PSUM"))
        dram = ctx.enter_context(tc.tile_pool(name="dram", bufs=1, space="DRAM"))

        # persistent operands
        imT8 = singles.tile([P, KC, IM_R], FP8)  # (-2*im)^T in fp8
        exT8s = [singles.tile([P, KC, GW], FP8, name=f"exT8_{j}") for j in range(N_JB)]
        imsq = singles.tile([P, N_IT], F32)
        cc = singles.tile([P, N_IT], F32)
        mv_im = singles.tile([P, N_IT, 2], F32)
        mv_dd = singles.tile([P, N_IT, 2], F32)
        exsq_cols = [singles.tile([P, EX_PER_JB], F32, name=f"exsqc{j}")
                     for j in range(N_JB)]
        acc_sb = singles.tile([P, N_G], F32)
        # exsq correction operands: psum[i, j] += 1*hi[j] + 1*lo[j] per bank,
        # replicated at partition bases 0/32/64/96 for row-tiled concurrent MMs.
        onesb = singles.tile([P, P], BF16)
        exrowb = singles.tile([P, N_JB * GW], BF16)
        exrow_dram = dram.tile([2, EX_R], BF16)

        nc.vector.memset(onesb, 1.0)

        def emit_ex_tile(t):
            jb, u = divmod(t, EX_PER_JB)
            ex_t = loads.tile([P, D], F32, tag="ex_t")
            nc.sync.dma_start(out=ex_t, in_=ex_d[t * P:(t + 1) * P, :])
            # exsq column via ACT square with fused accumulate
            sqo = scratch.tile([P, D], BF16, tag="sqo")
            nc.scalar.activation(out=sqo, in_=ex_t, func=AF.Square,
                                 accum_out=exsq_cols[jb][:, u:u + 1])
            # bf16 cast (GpSimd), XBAR transpose, then DVE cast to fp8
            exb = casts.tile([P, D], BF16, tag="exb")
            nc.gpsimd.tensor_copy(out=exb, in_=ex_t)
            tb = tbuf.tile([P, KC, P], BF16, tag="tb")
            nc.sync.dma_start_transpose(tb, exb)
            nc.vector.tensor_copy(out=exT8s[jb][:, :, u * P:(u + 1) * P], in_=tb)

        def emit_exrow_chunk(jb, b):
            # hi/lo bf16 split of exsq for bank b of group jb -> rows 0/1 of
            # exrowb (via DRAM scatter), then replicate to partitions 32/64/96.
            cols = exsq_cols[jb][:, 4 * b:4 * b + 4]
            hi = scratch.tile([P, 4], BF16, tag="hi")
            lo = scratch.tile([P, 4], BF16, tag="lo")
            nc.vector.tensor_copy(out=hi, in_=cols)
            nc.vector.tensor_tensor(out=lo, in0=cols, in1=hi, op=ALU.subtract)
            sl = slice(jb * GW + b * NJ, jb * GW + (b + 1) * NJ)
            nc.sync.dma_start(
                out=exrow_dram[0:1, sl].rearrange("o (u p) -> (o p) u", p=P), in_=hi)
            nc.sync.dma_start(
                out=exrow_dram[1:2, sl].rearrange("o (u p) -> (o p) u", p=P), in_=lo)
            nc.sync.dma_start(out=exrowb[0:2, sl], in_=exrow_dram[:, sl])
            for r in (32, 64, 96):
                nc.sync.dma_start(out=exrowb[r:r + 2, sl], in_=exrowb[0:2, sl])

        def emit_im_tile(t):
            im_t = loads.tile([P, D], F32, tag="im_t")
            s_t = loads.tile([P, D], F32, tag="s_t")
            nc.sync.dma_start(out=im_t, in_=im_d[t * P:(t + 1) * P, :])
            nc.sync.dma_start(out=s_t, in_=s_d[t * P:(t + 1) * P, :])
            # rowwise stats via bn_stats: sum(x^2) = D*(var + mean^2)
            st1 = scratch.tile([P, 6], F32, tag="st1")
            nc.vector.bn_stats(out=st1, in_=im_t)
            nc.vector.bn_aggr(out=mv_im[:, t, :], in_=st1)
            diff = scratch.tile([P, D], F32, tag="diff")
            nc.gpsimd.tensor_tensor(out=diff, in0=im_t, in1=s_t, op=ALU.subtract)
            st2 = scratch.tile([P, 6], F32, tag="st2")
            nc.vector.bn_stats(out=st2, in_=diff)
            nc.vector.bn_aggr(out=mv_dd[:, t, :], in_=st2)
            # per-tile finalize: imsq/dd columns available as soon as possible
            m2 = scratch.tile([P, 1], F32, tag="m2")
            nc.vector.tensor_tensor(out=m2, in0=mv_im[:, t, 0:1],
                                    in1=mv_im[:, t, 0:1], op=ALU.mult)
            nc.vector.tensor_tensor(out=m2, in0=m2, in1=mv_im[:, t, 1:2],
                                    op=ALU.add)
            nc.vector.tensor_scalar_mul(imsq[:, t:t + 1], m2, float(D))
            d2c = scratch.tile([P, 1], F32, tag="d2c")
            nc.vector.tensor_tensor(out=d2c, in0=mv_dd[:, t, 0:1],
                                    in1=mv_dd[:, t, 0:1], op=ALU.mult)
            nc.vector.tensor_tensor(out=d2c, in0=d2c, in1=mv_dd[:, t, 1:2],
                                    op=ALU.add)
            nc.vector.tensor_scalar_mul(d2c, d2c, float(D))
            ccs = scratch.tile([P, 1], F32, tag="ccs")
            nc.scalar.activation(out=ccs, in_=d2c, func=AF.Sqrt)
            nc.vector.tensor_scalar_add(cc[:, t:t + 1], ccs, MARGIN)
            # bf16 cast with the -2 fold, XBAR transpose, then cast to fp8
            imb = casts.tile([P, D], BF16, tag="imb")
            nc.vector.tensor_scalar_mul(imb, im_t, -2.0)
            tb = tbuf.tile([P, KC, P], BF16, tag="tbi")
            nc.sync.dma_start_transpose(tb, imb)
            nc.vector.tensor_copy(out=imT8[:, :, t * P:(t + 1) * P], in_=tb)

        # ---- preamble emission (order == scheduler priority == DMA order) ----
        def ex_chunk(jb, b):
            for u in range(4 * b, 4 * b + 4):
                emit_ex_tile(jb * EX_PER_JB + u)
            emit_exrow_chunk(jb, b)

        ex_chunk(0, 0)
        emit_im_tile(0)
        ex_chunk(0, 1)
        emit_im_tile(1)
        ex_chunk(0, 2)
        emit_im_tile(2)
        ex_chunk(0, 3)
        emit_im_tile(3)
        for t in range(4, 12):
            emit_im_tile(t)
        ex_chunk(1, 0)
        ex_chunk(1, 1)
        emit_im_tile(12)
        ex_chunk(1, 2)
        emit_im_tile(13)
        ex_chunk(1, 3)
        emit_im_tile(14)
        emit_im_tile(15)

        nc.sync.dma_start(out=cvec_d, in_=cc)

        # ---- main loop: 32 groups of [128 i, 2048 j] (4 PSUM banks each) ----
        for jb in range(N_JB):
            for it in range(N_IT):
                g = jb * N_IT + it
                ps = psum.tile([P, GW], F32, tag="mm")
                # exsq row adds: 4 concurrent row-tiled K=2 bf16 MMs (start)
                for b in range(BANKS):
                    r = 32 * b
                    nc.tensor.matmul(
                        ps[:, b * NJ:(b + 1) * NJ],
                        onesb[r:r + 2, :],
                        exrowb[r:r + 2, jb * GW + b * NJ:jb * GW + (b + 1) * NJ],
                        start=True, stop=False, tile_position=(r, 0))
                # fp8 DoubleRow data MMs: c-outer for stationary reuse
                for c in range(2):
                    for b in range(BANKS):
                        nc.tensor.matmul(
                            ps[:, b * NJ:(b + 1) * NJ],
                            imT8[:, 2 * c:2 * c + 2, it * P:(it + 1) * P],
                            exT8s[jb][:, 2 * c:2 * c + 2, b * NJ:(b + 1) * NJ],
                            start=False, stop=(c == 1), perf_mode=DR)
                # sq = sqrt(d2 + imsq[i])  (one ACT over all 4 banks, fp16 out)
                sq = sqp.tile([P, GW], FP16, tag="sq")
                nc.scalar.activation(out=sq, in_=ps, func=AF.Sqrt,
                                     bias=imsq[:, it:it + 1], scale=1.0)
                # acc[:, g] = sum_j min(sq, c)   (DVE, fused accumulate)
                nc.vector.tensor_scalar(sq, sq, cc[:, it:it + 1], 0.0,
                                        ALU.min, ALU.add,
                                        accum_out=acc_sb[:, g:g + 1])

        nc.sync.dma_start(out=acc_d, in_=acc_sb)


def build_program():
    nc = bacc.Bacc("TRN2", target_bir_lowering=False, debug=False)
    im_d = nc.dram_tensor("im", [IM_R, D], F32, kind="ExternalInput").ap()
    s_d = nc.dram_tensor("s", [IM_R, D], F32, kind="ExternalInput").ap()
    ex_d = nc.dram_tensor("ex", [EX_R, D], F32, kind="ExternalInput").ap()
    acc_d = nc.dram_tensor("acc", [P, N_G], F32, kind="ExternalOutput").ap()
    cvec_d = nc.dram_tensor("cvec", [P, N_IT], F32, kind="ExternalOutput").ap()
    with tile.TileContext(nc) as tc:
        _emit(tc, nc, im_d, s_d, ex_d, acc_d, cvec_d)
    nc.compile()
    return nc


def get_program():
    if "nc" not in _CACHE:
        _CACHE["nc"] = build_program()
    return _CACHE["nc"]


def make_in_maps(im, s, ex_s):
    in_maps = []
    for c in range(8):
        ig, jg = divmod(c, J_GROUPS)
        in_maps.append({
            "im": np.ascontiguousarray(im[ig * IM_R:(ig + 1) * IM_R], dtype=np.float32),
            "s": np.ascontiguousarray(s[ig * IM_R:(ig + 1) * IM_R], dtype=np.float32),
            "ex": np.ascontiguousarray(ex_s[jg * EX_R:(jg + 1) * EX_R], dtype=np.float32),
        })
    return in_maps


def finish(results):
    # per core: sum_ij relu(c_i - sq_ij) = EX_R * sum_i c_i - sum_ij min(sq, c)
    total = 0.0
    for r in results:
        total += float(EX_R) * float(np.sum(r["cvec"], dtype=np.float64))
        total -= float(np.sum(r["acc"], dtype=np.float64))
    return np.array(total / (float(N) * float(N)), dtype=np.float32)


def kernel(im, s, ex_s):
    nc = get_program()
    res = bass_utils.run_bass_kernel_spmd(nc, make_in_maps(im, s, ex_s),
                                          core_ids=list(range(8)))
    return finish(res.results)


if __name__ == "__main__":
    rng = np.random.default_rng(0)
    im = rng.standard_normal((N, D), dtype=np.float32)
    s = rng.standard_normal((N, D), dtype=np.float32)
    ex = rng.standard_normal((N, D), dtype=np.float32)
    print(kernel(im, s, ex))


# revision 3
# speedup vs baseline: 1.8175x; 1.8175x over previous
"""Trainium2 Bass kernel: CapOnlyContrastiveLoss (margin contrastive loss, mean reduction).

reference math (N=8192, D=512, margin=0.2):
    scores[i,j]  = -||im_i - ex_j||        (via gemm identity)
    diag[i]      = -||im_i - s_i||         (only the diagonal of l2_sim(im, s) is used)
    loss         = mean(relu(margin + scores - diag))

Strategy (v3):
  * 4x2 core grid: each core 2048 im/s rows x 4096 ex rows (16 MB HBM).
  * PE: -2*im.ex via fp8(e4m3) DoubleRow MMs (K=512 in 2 MMs of K=256) plus a
    K=2 bf16 exsq-fold MM per bank, row-tiled (tile_position 0/32/64/96) so the
    4 per-group exsq MMs run concurrently.
  * Loads batched (ex quads, im/s pairs) to cut SP descriptor work; XBAR
    transposes batched likewise.  fp32 -> bf16 on DVE (2x_2P), transposed bf16
    -> fp8 on GpSimd (ex) / DVE (im).
  * All row stats via ACT Square with fused accumulate (imsq, dd, exsq).
  * Epilogue per (jb,it) group of 4 PSUM banks:
      ACT: sq[128,2048] = sqrt(psum + imsq[i])  -> bf16
      then either DVE tensor_scalar (min c, +acc) or ACT relu(c - sq) accum.
  * Host: loss = sum_cores(per-group terms) / N^2.
"""

import numpy as np

import concourse.bacc as bacc
import concourse.bass as bass
import concourse.tile as tile
from concourse import bass_utils, mybir

N, D = 8192, 512
MARGIN = 0.2
P = 128
NJ = 512  # one PSUM bank of fp32
GW = 2048  # epilogue group width along j (4 PSUM banks)
I_GROUPS, J_GROUPS = 4, 2  # 8 cores
IM_R = N // I_GROUPS  # 2048
EX_R = N // J_GROUPS  # 4096
KC = D // P  # 4
N_IT = IM_R // P  # 16
N_JB = EX_R // GW  # 2
BANKS = GW // NJ  # 4
EX_PER_JB = GW // P  # 16
N_G = N_JB * N_IT  # 32

# epilogue groups finished on ACT (relu accum) instead of DVE (min accum)
GROUPS_ON_ACT = frozenset()

F32 = mybir.dt.float32
BF16 = mybir.dt.bfloat16
FP8 = mybir.dt.float8e4
AF = mybir.ActivationFunctionType
ALU = mybir.AluOpType
DR = mybir.MatmulPerfMode.DoubleRow

_CACHE = {}


def _emit(tc, nc, im_d, s_d, ex_d, acc_d, cvec_d):
    from contextlib import ExitStack

    with ExitStack() as ctx:
        singles = ctx.enter_context(tc.tile_pool(name="singles", bufs=1))
        exl = ctx.enter_context(tc.tile_pool(name="exl", bufs=3))
        iml = ctx.enter_context(tc.tile_pool(name="iml", bufs=3))
        casts = ctx.enter_context(tc.tile_pool(name="casts", bufs=4))
        tbuf = ctx.enter_context(tc.tile_pool(name="tbuf", bufs=3))
        scratch = ctx.enter_context(tc.tile_pool(name="scratch", bufs=8))
        sqp = ctx.enter_context(tc.tile_pool(name="sqp", bufs=2))
        psum = ctx.enter_context(tc.tile_pool(name="psum", bufs=2, space="PSUM"))
        dram = ctx.enter_context(tc.tile_pool(name="dram", bufs=1, space="DRAM"))

        imT8 = singles.tile([P, KC, IM_R], FP8)
        exT8s = [singles.tile([P, KC, GW], FP8, name=f"exT8_{j}") for j in range(N_JB)]
        imsq = singles.tile([P, N_IT], F32)
        ddv = singles.tile([P, N_IT], F32)
        cc = singles.tile([P, N_IT], F32)
        exsq_cols = [singles.tile([P, EX_PER_JB], F32, name=f"exsqc{j}")
                     for j in range(N_JB)]
        acc_sb = singles.tile([P, N_G], F32)
        onesb = singles.tile([P, P], BF16)
        exrowb = singles.tile([P, N_JB * GW], BF16)
        exrow_dram = dram.tile([2, EX_R], BF16)

        nc.vector.memset(onesb, 1.0)

        def emit_ex_quad(jb, q):
            # one DMA for 4 ex tiles; per tile: DVE bf16 cast + ACT square;
            # one XBAR transpose of the quad; GpSimd fp8 casts per pair.
            u0 = 4 * q
            t0 = jb * EX_PER_JB + u0
            exq = exl.tile([P, 4, D], F32, tag="exq")
            nc.sync.dma_start(
                out=exq,
                in_=ex_d[t0 * P:(t0 + 4) * P, :].rearrange("(t p) d -> p t d", p=P))
            exb = casts.tile([P, 4, D], BF16, tag="exb")
            for u in range(4):
                nc.scalar.activation(
                    out=scratch.tile([P, D], BF16, tag="sqo", name="sqo"), in_=exq[:, u, :],
                    func=AF.Square,
                    accum_out=exsq_cols[jb][:, u0 + u:u0 + u + 1])
                nc.vector.tensor_copy(out=exb[:, u, :], in_=exq[:, u, :])
            tb = tbuf.tile([P, 4 * KC, P], BF16, tag="tb")
            nc.sync.dma_start_transpose(tb, exb)
            for h in range(2):
                dst = exT8s[jb][:, :, (u0 + 2 * h) * P:(u0 + 2 * h + 2) * P]
                nc.gpsimd.tensor_copy(
                    out=dst.rearrange("p k (t f) -> p k t f", t=2),
                    in_=tb[:, 8 * h:8 * h + 8, :].rearrange("p (t k) f -> p k t f", t=2))

        def emit_exrow(jb):
            # hi/lo bf16 split of exsq -> rows 0/1 (DRAM scatter), replicate to
            # partition bases 32/64/96 for the row-tiled exsq MMs.
            cols = exsq_cols[jb]
            hi = scratch.tile([P, EX_PER_JB], BF16, tag="hi")
            lo = scratch.tile([P, EX_PER_JB], BF16, tag="lo")
            nc.vector.tensor_copy(out=hi, in_=cols)
            nc.vector.tensor_tensor(out=lo, in0=cols, in1=hi, op=ALU.subtract)
            sl = slice(jb * GW, (jb + 1) * GW)
            nc.sync.dma_start(
                out=exrow_dram[0:1, sl].rearrange("o (u p) -> (o p) u", p=P), in_=hi)
            nc.sync.dma_start(
                out=exrow_dram[1:2, sl].rearrange("o (u p) -> (o p) u", p=P), in_=lo)
            nc.sync.dma_start(out=exrowb[0:2, sl], in_=exrow_dram[:, sl])
            for r in (32, 64, 96):
                nc.sync.dma_start(out=exrowb[r:r + 2, sl], in_=exrowb[0:2, sl])

        def emit_im_pair(k):
            t0 = 2 * k
            imp = iml.tile([P, 2, D], F32, tag="imp")
            sp = iml.tile([P, 2, D], F32, tag="sp")
            nc.sync.dma_start(
                out=imp,
                in_=im_d[t0 * P:(t0 + 2) * P, :].rearrange("(t p) d -> p t d", p=P))
            nc.sync.dma_start(
                out=sp,
                in_=s_d[t0 * P:(t0 + 2) * P, :].rearrange("(t p) d -> p t d", p=P))
            imb = casts.tile([P, 2, D], BF16, tag="imb")
            for u in range(2):
                t = t0 + u
                nc.scalar.activation(
                    out=scratch.tile([P, D], BF16, tag="dmp", name="dmp"), in_=imp[:, u, :],
                    func=AF.Square, accum_out=imsq[:, t:t + 1])
                diff = scratch.tile([P, D], F32, tag="diff")
                nc.gpsimd.tensor_tensor(out=diff, in0=imp[:, u, :], in1=sp[:, u, :],
                                        op=ALU.subtract)
                nc.scalar.activation(
                    out=scratch.tile([P, D], BF16, tag="dmp2", name="dmp2"), in_=diff,
                    func=AF.Square, accum_out=ddv[:, t:t + 1])
                nc.vector.tensor_scalar_mul(imb[:, u, :], imp[:, u, :], -2.0)
            tb = tbuf.tile([P, 2 * KC, P], BF16, tag="tbi")
            nc.sync.dma_start_transpose(tb, imb)
            dst = imT8[:, :, t0 * P:(t0 + 2) * P]
            nc.vector.tensor_copy(
                out=dst.rearrange("p k (t f) -> p k t f", t=2),
                in_=tb.rearrange("p (t k) f -> p k t f", t=2))

        def emit_cc(b4):
            # cc[:, 4b:4b+4] = margin + sqrt(dd) for a batch of 4 tiles
            sl = slice(4 * b4, 4 * b4 + 4)
            ccs = scratch.tile([P, 4], F32, tag="ccs")
            nc.scalar.activation(out=ccs, in_=ddv[:, sl], func=AF.Sqrt)
            nc.vector.tensor_scalar_add(cc[:, sl], ccs, MARGIN)

        # ---- preamble (emission order == priority == SP queue order) ----
        emit_ex_quad(0, 0)
        emit_ex_quad(0, 1)
        emit_im_pair(0)
        emit_ex_quad(0, 2)
        emit_im_pair(1)
        emit_ex_quad(0, 3)
        emit_exrow(0)
        emit_cc(0)
        emit_im_pair(2)
        emit_im_pair(3)
        emit_cc(1)
        emit_im_pair(4)
        emit_im_pair(5)
        emit_cc(2)
        emit_ex_quad(1, 0)
        emit_im_pair(6)
        emit_ex_quad(1, 1)
        emit_im_pair(7)
        emit_cc(3)
        emit_ex_quad(1, 2)
        emit_ex_quad(1, 3)
        emit_exrow(1)
        nc.sync.dma_start(out=cvec_d, in_=cc)

        # ---- main loop: 32 groups of [128 i, 2048 j] ----
        for jb in range(N_JB):
            for it in range(N_IT):
                g = jb * N_IT + it
                ps = psum.tile([P, GW], F32, tag="mm")

                def mm_exsq(start, stop):
                    for b in range(BANKS):
                        r = 32 * b
                        nc.tensor.matmul(
                            ps[:, b * NJ:(b + 1) * NJ],
                            onesb[r:r + 2, :],
                            exrowb[r:r + 2, jb * GW + b * NJ:jb * GW + (b + 1) * NJ],
                            start=start, stop=stop, tile_position=(r, 0))

                def mm_dr(c, start, stop):
                    for b in range(BANKS):
                        nc.tensor.matmul(
                            ps[:, b * NJ:(b + 1) * NJ],
                            imT8[:, 2 * c:2 * c + 2, it * P:(it + 1) * P],
                            exT8s[jb][:, 2 * c:2 * c + 2, b * NJ:(b + 1) * NJ],
                            start=start, stop=stop, perf_mode=DR)

                if g < 2:
                    # first groups: data MMs first so PE can start before the
                    # exsq scatter round-trip completes
                    mm_dr(0, True, False)
                    mm_dr(1, False, False)
                    mm_exsq(False, True)
                else:
                    mm_exsq(True, False)
                    mm_dr(0, False, False)
                    mm_dr(1, False, True)

                sq = sqp.tile([P, GW], BF16, tag="sq")
                nc.scalar.activation(out=sq, in_=ps, func=AF.Sqrt,
                                     bias=imsq[:, it:it + 1], scale=1.0)
                if g in GROUPS_ON_ACT:
                    # acc[:, g] = sum_j relu(c - sq)   (ACT, fused accumulate)
                    nc.scalar.activation(
                        out=sqp.tile([P, GW], BF16, tag="dump", name="dump"), in_=sq,
                        func=AF.Relu, bias=cc[:, it:it + 1], scale=-1.0,
                        accum_out=acc_sb[:, g:g + 1])
                else:
                    # acc[:, g] = sum_j min(sq, c)   (DVE, fused accumulate)
                    mout = sqp.tile([P, GW], BF16, tag="mout")
                    nc.vector.tensor_scalar(mout, sq, cc[:, it:it + 1], 0.0,
                                            ALU.min, ALU.add,
                                            accum_out=acc_sb[:, g:g + 1])

        nc.sync.dma_start(out=acc_d, in_=acc_sb)


def build_program():
    nc = bacc.Bacc("TRN2", target_bir_lowering=False, debug=False)
    im_d = nc.dram_tensor("im", [IM_R, D], F32, kind="ExternalInput").ap()
    s_d = nc.dram_tensor("s", [IM_R, D], F32, kind="ExternalInput").ap()
    ex_d = nc.dram_tensor("ex", [EX_R, D], F32, kind="ExternalInput").ap()
    acc_d = nc.dram_tensor("acc", [P, N_G], F32, kind="ExternalOutput").ap()
    cvec_d = nc.dram_tensor("cvec", [P, N_IT], F32, kind="ExternalOutput").ap()
    with tile.TileContext(nc) as tc:
        _emit(tc, nc, im_d, s_d, ex_d, acc_d, cvec_d)
    nc.compile()
    return nc


def get_program():
    if "nc" not in _CACHE:
        _CACHE["nc"] = build_program()
    return _CACHE["nc"]


def make_in_maps(im, s, ex_s):
    in_maps = []
    for c in range(8):
        ig, jg = divmod(c, J_GROUPS)
        in_maps.append({
            "im": np.ascontiguousarray(im[ig * IM_R:(ig + 1) * IM_R], dtype=np.float32),
            "s": np.ascontiguousarray(s[ig * IM_R:(ig + 1) * IM_R], dtype=np.float32),
            "ex": np.ascontiguousarray(ex_s[jg * EX_R:(jg + 1) * EX_R], dtype=np.float32),
        })
    return in_maps


def finish(results):
    # DVE groups: sum_j relu(c_i - sq) = GW * c_i - sum_j min(sq, c)
    # ACT groups: acc is already sum_j relu(c_i - sq)
    total = 0.0
    for r in results:
        cvec = np.asarray(r["cvec"], dtype=np.float64)  # [P, N_IT]
        acc = np.asarray(r["acc"], dtype=np.float64)  # [P, N_G]
        csum = cvec.sum(axis=0)  # per i-tile sums
        for g in range(N_G):
            it = g % N_IT
            if g in GROUPS_ON_ACT:
                total += acc[:, g].sum()
            else:
                total += GW * csum[it] - acc[:, g].sum()
    return np.array(total / (float(N) * float(N)), dtype=np.float32)


def kernel(im, s, ex_s):
    nc = get_program()
    res = bass_utils.run_bass_kernel_spmd(nc, make_in_maps(im, s, ex_s),
                                          core_ids=list(range(8)))
    return finish(res.results)


if __name__ == "__main__":
    rng = np.random.default_rng(0)
    im = rng.standard_normal((N, D), dtype=np.float32)
    s = rng.standard_normal((N, D), dtype=np.float32)
    ex = rng.standard_normal((N, D), dtype=np.float32)
    print(kernel(im, s, ex))


# revision 4
# speedup vs baseline: 2.1519x; 1.1840x over previous
"""Trainium2 Bass kernel: CapOnlyContrastiveLoss (margin contrastive loss, mean reduction).

reference math (N=8192, D=512, margin=0.2):
    scores[i,j]  = -||im_i - ex_j||        (via gemm identity)
    diag[i]      = -||im_i - s_i||         (only diag of l2_sim(im, s) is used)
    loss         = mean(relu(margin + scores - diag))

Strategy (v4): 4x2 core grid, fp8 DoubleRow MMs + row-tiled bf16 exsq-fold MMs,
grouped 4-bank epilogue.  vs v3: exsq scatter DMAs issued from the GpSimd DGE
(no SP head-of-line blocking), whole-quad DVE bf16 casts, fp8 casts on DVE,
im/dd stats via DVE bn_stats, and a 3-way epilogue split (DVE STT / DVE
tensor_scalar+accum / ACT relu+accum) to compare engine costs on HW.
"""

import numpy as np

import concourse.bacc as bacc
import concourse.bass as bass
import concourse.tile as tile
from concourse import bass_utils, mybir

N, D = 8192, 512
MARGIN = 0.2
P = 128
NJ = 512
GW = 2048
I_GROUPS, J_GROUPS = 4, 2
IM_R = N // I_GROUPS  # 2048
EX_R = N // J_GROUPS  # 4096
KC = D // P  # 4
N_IT = IM_R // P  # 16
N_JB = EX_R // GW  # 2
BANKS = GW // NJ  # 4
EX_PER_JB = GW // P  # 16
N_G = N_JB * N_IT  # 32

# per-group epilogue kind: 'stt' (DVE scalar_tensor_tensor min+acc),
# 'cache' (DVE tensor_scalar min+acc), 'act' (ACT relu(c-sq)+acc)
GROUP_KIND = ['stt'] * 16 + ['cache'] * 12 + ['act'] * 4

F32 = mybir.dt.float32
BF16 = mybir.dt.bfloat16
FP8 = mybir.dt.float8e4
AF = mybir.ActivationFunctionType
ALU = mybir.AluOpType
DR = mybir.MatmulPerfMode.DoubleRow

_CACHE = {}


def _emit(tc, nc, im_d, s_d, ex_d, acc_d, cvec_d):
    from contextlib import ExitStack

    with ExitStack() as ctx:
        singles = ctx.enter_context(tc.tile_pool(name="singles", bufs=1))
        exl = ctx.enter_context(tc.tile_pool(name="exl", bufs=3))
        iml = ctx.enter_context(tc.tile_pool(name="iml", bufs=3))
        casts = ctx.enter_context(tc.tile_pool(name="casts", bufs=3))
        tbuf = ctx.enter_context(tc.tile_pool(name="tbuf", bufs=3))
        scratch = ctx.enter_context(tc.tile_pool(name="scratch", bufs=6))
        sqp = ctx.enter_context(tc.tile_pool(name="sqp", bufs=2))
        psum = ctx.enter_context(tc.tile_pool(name="psum", bufs=2, space="PSUM"))
        dram = ctx.enter_context(tc.tile_pool(name="dram", bufs=1, space="DRAM"))

        imT8 = singles.tile([P, KC, IM_R], FP8)
        exT8s = [singles.tile([P, KC, GW], FP8, name=f"exT8_{j}") for j in range(N_JB)]
        imsq = singles.tile([P, N_IT], F32)
        cc = singles.tile([P, N_IT], F32)
        mv_im = singles.tile([P, N_IT, 2], F32)
        mv_dd = singles.tile([P, N_IT, 2], F32)
        exsq_cols = [singles.tile([P, EX_PER_JB], F32, name=f"exsqc{j}")
                     for j in range(N_JB)]
        acc_sb = singles.tile([P, N_G], F32)
        onesb = singles.tile([P, P], BF16)
        zerosb = singles.tile([P, GW], BF16)
        exrowb = singles.tile([P, N_JB * GW], BF16)
        exrow_dram = dram.tile([2, EX_R], BF16)

        nc.vector.memset(onesb, 1.0)
        nc.vector.memset(zerosb, 0.0)

        def emit_ex_quad(jb, q):
            u0 = 4 * q
            t0 = jb * EX_PER_JB + u0
            exq = exl.tile([P, 4, D], F32, tag="exq")
            nc.sync.dma_start(
                out=exq,
                in_=ex_d[t0 * P:(t0 + 4) * P, :].rearrange("(t p) d -> p t d", p=P))
            # exsq columns via ACT square with fused accumulate (per tile)
            for u in range(4):
                nc.scalar.activation(
                    out=scratch.tile([P, D], BF16, tag="sqo", name="sqo"),
                    in_=exq[:, u, :], func=AF.Square,
                    accum_out=exsq_cols[jb][:, u0 + u:u0 + u + 1])
            # whole-quad bf16 cast (DVE 2x_2P), XBAR transpose, fp8 casts (DVE)
            exb = casts.tile([P, 4, D], BF16, tag="exb")
            nc.vector.tensor_copy(out=exb, in_=exq)
            tb = tbuf.tile([P, 4 * KC, P], BF16, tag="tb")
            nc.sync.dma_start_transpose(tb, exb)
            for h in range(2):
                dst = exT8s[jb][:, :, (u0 + 2 * h) * P:(u0 + 2 * h + 2) * P]
                nc.vector.tensor_copy(
                    out=dst.rearrange("p k (t f) -> p k t f", t=2),
                    in_=tb[:, 8 * h:8 * h + 8, :].rearrange("p (t k) f -> p k t f", t=2))

        def emit_exrow(jb):
            # hi/lo bf16 split of exsq -> rows 0/1 via DRAM scatter; all these
            # DMAs ride the GpSimd DGE so the SP load queue never blocks.
            cols = exsq_cols[jb]
            hi = scratch.tile([P, EX_PER_JB], BF16, tag="hi")
            lo = scratch.tile([P, EX_PER_JB], BF16, tag="lo")
            nc.vector.tensor_copy(out=hi, in_=cols)
            nc.vector.tensor_tensor(out=lo, in0=cols, in1=hi, op=ALU.subtract)
            sl = slice(jb * GW, (jb + 1) * GW)
            nc.gpsimd.dma_start(
                out=exrow_dram[0:1, sl].rearrange("o (u p) -> (o p) u", p=P), in_=hi)
            nc.gpsimd.dma_start(
                out=exrow_dram[1:2, sl].rearrange("o (u p) -> (o p) u", p=P), in_=lo)
            nc.gpsimd.dma_start(out=exrowb[0:2, sl], in_=exrow_dram[:, sl])
            for r in (32, 64, 96):
                nc.gpsimd.dma_start(out=exrowb[r:r + 2, sl], in_=exrowb[0:2, sl])

        def emit_im_pair(k):
            t0 = 2 * k
            imp = iml.tile([P, 2, D], F32, tag="imp")
            sp = iml.tile([P, 2, D], F32, tag="sp")
            nc.sync.dma_start(
                out=imp,
                in_=im_d[t0 * P:(t0 + 2) * P, :].rearrange("(t p) d -> p t d", p=P))
            nc.sync.dma_start(
                out=sp,
                in_=s_d[t0 * P:(t0 + 2) * P, :].rearrange("(t p) d -> p t d", p=P))
            imb = casts.tile([P, 2, D], BF16, tag="imb")
            nc.vector.tensor_scalar_mul(imb, imp, -2.0)
            for u in range(2):
                t = t0 + u
                st1 = scratch.tile([P, 6], F32, tag="st1")
                nc.vector.bn_stats(out=st1, in_=imp[:, u, :])
                nc.vector.bn_aggr(out=mv_im[:, t, :], in_=st1)
                diff = scratch.tile([P, D], F32, tag="diff")
                nc.gpsimd.tensor_tensor(out=diff, in0=imp[:, u, :], in1=sp[:, u, :],
                                        op=ALU.subtract)
                st2 = scratch.tile([P, 6], F32, tag="st2")
                nc.vector.bn_stats(out=st2, in_=diff)
                nc.vector.bn_aggr(out=mv_dd[:, t, :], in_=st2)
            tb = tbuf.tile([P, 2 * KC, P], BF16, tag="tbi")
            nc.sync.dma_start_transpose(tb, imb)
            dst = imT8[:, :, t0 * P:(t0 + 2) * P]
            nc.vector.tensor_copy(
                out=dst.rearrange("p k (t f) -> p k t f", t=2),
                in_=tb.rearrange("p (t k) f -> p k t f", t=2))

        def emit_fin(b4):
            # batched finalize for tiles [4b, 4b+4): imsq, dd, cc
            sl = slice(4 * b4, 4 * b4 + 4)
            for mv, dest in ((mv_im, imsq), (mv_dd, None)):
                t4 = scratch.tile([P, 4], F32, tag="t4", name="t4")
                nc.vector.tensor_tensor(out=t4, in0=mv[:, sl, 0], in1=mv[:, sl, 0],
                                        op=ALU.mult)
                nc.vector.tensor_tensor(out=t4, in0=t4, in1=mv[:, sl, 1], op=ALU.add)
                if dest is not None:
                    nc.vector.tensor_scalar_mul(dest[:, sl], t4, float(D))
                else:
                    nc.vector.tensor_scalar_mul(t4, t4, float(D))
                    ccs = scratch.tile([P, 4], F32, tag="ccs")
                    nc.scalar.activation(out=ccs, in_=t4, func=AF.Sqrt)
                    nc.vector.tensor_scalar_add(cc[:, sl], ccs, MARGIN)

        # ---- preamble ----
        emit_ex_quad(0, 0)
        emit_ex_quad(0, 1)
        emit_im_pair(0)
        emit_ex_quad(0, 2)
        emit_im_pair(1)
        emit_ex_quad(0, 3)
        emit_exrow(0)
        emit_fin(0)
        emit_im_pair(2)
        emit_im_pair(3)
        emit_fin(1)
        emit_im_pair(4)
        emit_im_pair(5)
        emit_fin(2)
        emit_ex_quad(1, 0)
        emit_im_pair(6)
        emit_ex_quad(1, 1)
        emit_im_pair(7)
        emit_fin(3)
        emit_ex_quad(1, 2)
        emit_ex_quad(1, 3)
        emit_exrow(1)
        nc.sync.dma_start(out=cvec_d, in_=cc)

        # ---- main loop ----
        for jb in range(N_JB):
            for it in range(N_IT):
                g = jb * N_IT + it
                ps = psum.tile([P, GW], F32, tag="mm")

                def mm_exsq(start, stop):
                    for b in range(BANKS):
                        r = 32 * b
                        nc.tensor.matmul(
                            ps[:, b * NJ:(b + 1) * NJ],
                            onesb[r:r + 2, :],
                            exrowb[r:r + 2, jb * GW + b * NJ:jb * GW + (b + 1) * NJ],
                            start=start, stop=stop, tile_position=(r, 0))

                def mm_dr(c, start, stop):
                    for b in range(BANKS):
                        nc.tensor.matmul(
                            ps[:, b * NJ:(b + 1) * NJ],
                            imT8[:, 2 * c:2 * c + 2, it * P:(it + 1) * P],
                            exT8s[jb][:, 2 * c:2 * c + 2, b * NJ:(b + 1) * NJ],
                            start=start, stop=stop, perf_mode=DR)

                if g < 2:
                    mm_dr(0, True, False)
                    mm_dr(1, False, False)
                    mm_exsq(False, True)
                else:
                    mm_exsq(True, False)
                    mm_dr(0, False, False)
                    mm_dr(1, False, True)

                sq = sqp.tile([P, GW], BF16, tag="sq")
                nc.scalar.activation(out=sq, in_=ps, func=AF.Sqrt,
                                     bias=imsq[:, it:it + 1], scale=1.0)
                kind = GROUP_KIND[g]
                mout = sqp.tile([P, GW], BF16, tag="mout")
                if kind == 'act':
                    nc.scalar.activation(
                        out=mout, in_=sq, func=AF.Relu,
                        bias=cc[:, it:it + 1], scale=-1.0,
                        accum_out=acc_sb[:, g:g + 1])
                elif kind == 'stt':
                    nc.vector.scalar_tensor_tensor(
                        out=mout, in0=sq, scalar=cc[:, it:it + 1], in1=zerosb,
                        op0=ALU.min, op1=ALU.add,
                        accum_out=acc_sb[:, g:g + 1])
                else:
                    nc.vector.tensor_scalar(mout, sq, cc[:, it:it + 1], 0.0,
                                            ALU.min, ALU.add,
                                            accum_out=acc_sb[:, g:g + 1])

        nc.sync.dma_start(out=acc_d, in_=acc_sb)


def build_program():
    nc = bacc.Bacc("TRN2", target_bir_lowering=False, debug=False)
    im_d = nc.dram_tensor("im", [IM_R, D], F32, kind="ExternalInput").ap()
    s_d = nc.dram_tensor("s", [IM_R, D], F32, kind="ExternalInput").ap()
    ex_d = nc.dram_tensor("ex", [EX_R, D], F32, kind="ExternalInput").ap()
    acc_d = nc.dram_tensor("acc", [P, N_G], F32, kind="ExternalOutput").ap()
    cvec_d = nc.dram_tensor("cvec", [P, N_IT], F32, kind="ExternalOutput").ap()
    with tile.TileContext(nc) as tc:
        _emit(tc, nc, im_d, s_d, ex_d, acc_d, cvec_d)
    nc.compile()
    return nc


def get_program():
    if "nc" not in _CACHE:
        _CACHE["nc"] = build_program()
    return _CACHE["nc"]


def make_in_maps(im, s, ex_s):
    in_maps = []
    for c in range(8):
        ig, jg = divmod(c, J_GROUPS)
        in_maps.append({
            "im": np.ascontiguousarray(im[ig * IM_R:(ig + 1) * IM_R], dtype=np.float32),
            "s": np.ascontiguousarray(s[ig * IM_R:(ig + 1) * IM_R], dtype=np.float32),
            "ex": np.ascontiguousarray(ex_s[jg * EX_R:(jg + 1) * EX_R], dtype=np.float32),
        })
    return in_maps


def finish(results):
    total = 0.0
    for r in results:
        cvec = np.asarray(r["cvec"], dtype=np.float64)
        acc = np.asarray(r["acc"], dtype=np.float64)
        csum = cvec.sum(axis=0)
        for g in range(N_G):
            it = g % N_IT
            if GROUP_KIND[g] == 'act':
                total += acc[:, g].sum()
            else:
                total += GW * csum[it] - acc[:, g].sum()
    return np.array(total / (float(N) * float(N)), dtype=np.float32)


def kernel(im, s, ex_s):
    nc = get_program()
    res = bass_utils.run_bass_kernel_spmd(nc, make_in_maps(im, s, ex_s),
                                          core_ids=list(range(8)))
    return finish(res.results)


if __name__ == "__main__":
    rng = np.random.default_rng(0)
    im = rng.standard_normal((N, D), dtype=np.float32)
    s = rng.standard_normal((N, D), dtype=np.float32)
    ex = rng.standard_normal((N, D), dtype=np.float32)
    print(kernel(im, s, ex))
